# revision 1
# baseline (speedup 1.0000x reference)
"""NetVLAD Trainium2 kernel: data-parallel over batch across 8 NeuronCores."""
import sys
sys.path.insert(0, "/opt/trn_rl_repo")
import os
import numpy as np
import concourse.bass as bass
import concourse.tile as tile
from concourse import bacc, mybir, masks
from concourse import bass_utils

F32 = mybir.dt.float32
N, C, HW, K = 32, 512, 1600, 64
NCORES = 8
NPC = N // NCORES          # images per core
EPS = 1e-12
CT = C // 128              # 4 c-tiles
CHUNKS = [(0, 512), (512, 512), (1024, 512), (1536, 64)]
SKIP_VLAD = os.environ.get("SKIP_VLAD") == "1"
SKIP_ASUM = os.environ.get("SKIP_ASUM") == "1"
SKIP_TOT = os.environ.get("SKIP_TOT") == "1"
SKIP_TAIL = os.environ.get("SKIP_TAIL") == "1"

_CACHE = {}


def _build():
    nc = bacc.Bacc("TRN2", target_bir_lowering=False, debug=False, num_devices=NCORES)
    x_d = nc.dram_tensor("x", [NPC, C, HW], F32, kind="ExternalInput")
    w_d = nc.dram_tensor("conv_w", [K, C], F32, kind="ExternalInput")
    b_d = nc.dram_tensor("conv_b", [1, K], F32, kind="ExternalInput")
    c_d = nc.dram_tensor("centroids", [K, C], F32, kind="ExternalInput")
    y_d = nc.dram_tensor("y", [NPC, K * C], F32, kind="ExternalOutput")

    with tile.TileContext(nc) as tc:
        _emit(nc, tc, x_d, w_d, b_d, c_d, y_d)
    nc.finalize()
    return nc


def _emit(nc, tc, x_d, w_d, b_d, c_d, y_d):
    import contextlib
    ctx = contextlib.ExitStack()
    with ctx:
        const = ctx.enter_context(tc.tile_pool(name="const", bufs=1))
        xp = ctx.enter_context(tc.tile_pool(name="xp", bufs=2))
        sb = ctx.enter_context(tc.tile_pool(name="sb", bufs=2))
        ps = ctx.enter_context(tc.tile_pool(name="ps", bufs=2, space="PSUM"))

        ident = const.tile([128, 128], F32)
        masks.make_identity(nc, ident[:])
        ones64 = const.tile([64, 64], F32)
        nc.vector.memset(ones64[:], 1.0)
        cent = const.tile([64, C], F32)
        nc.sync.dma_start(cent[:], c_d[:, :])
        # b broadcast to all partitions, replicated 4x along free dim
        b_bc = const.tile([128, 256], F32)
        for s in range(4):
            nc.sync.dma_start(b_bc[:, s * 64:(s + 1) * 64],
                              b_d[0:1, :].broadcast_to([128, K]))
        # conv_w^T tiles: load conv_w [64, 512], transpose 4 blocks -> [128, 64] each
        w_sb = const.tile([64, C], F32)
        nc.sync.dma_start(w_sb[:], w_d[:, :])
        wT = const.tile([128, CT * 64], F32)
        for ct in range(CT):
            tp = ps.tile([128, 64], F32, tag="lgt")
            nc.tensor.transpose(tp[:], w_sb[:, ct * 128:(ct + 1) * 128],
                                ident[0:64, 0:64])
            nc.scalar.copy(wT[:, ct * 64:(ct + 1) * 64], tp[:])

        for n in range(NPC):
            xs = [xp.tile([128, HW], F32, tag=f"x{ct}", name=f"x{ct}_{n}")
                  for ct in range(CT)]
            for ct in range(CT):
                nc.sync.dma_start(xs[ct][:], x_d[n, ct * 128:(ct + 1) * 128, :])

            vacc = sb.tile([64, C], F32, tag="vacc", name=f"vacc{n}")
            aacc = sb.tile([64, 1], F32, tag="aacc", name=f"aacc{n}")

            my_chunks = CHUNKS[:3] if SKIP_TAIL else CHUNKS
            for lc, (l0, W) in enumerate(my_chunks):
                S = W // 128 if W >= 128 else 1     # segments of <=128 pixels
                P = 128 if W >= 128 else W          # pixels per segment
                first = lc == 0
                last = lc == len(my_chunks) - 1

                # ---- logits [K, W] ----
                lg = ps.tile([64, W], F32, tag="lg")
                for ct in range(CT):
                    nc.tensor.matmul(lg[:], wT[:, ct * 64:(ct + 1) * 64],
                                     xs[ct][:, l0:l0 + W],
                                     start=(ct == 0), stop=(ct == CT - 1))
                lg_sb = sb.tile([64, W], F32, tag="lg_sb")
                nc.scalar.copy(lg_sb[:], lg[:])

                # ---- logits^T [P, S*64] ----
                lgt = ps.tile([P, S * 64], F32, tag="lgt")
                for s in range(S):
                    nc.tensor.transpose(lgt[:, s * 64:(s + 1) * 64],
                                        lg_sb[:, s * P:(s + 1) * P],
                                        ident[0:64, 0:64])

                # ---- x^T per segment + sumsq ----
                ss = sb.tile([P, S], F32, tag="ss")
                xts = []
                for s in range(S):
                    xt_ps = ps.tile([P, C], F32, tag="xt")
                    for ct in range(CT):
                        nc.tensor.transpose(xt_ps[:, ct * 128:(ct + 1) * 128],
                                            xs[ct][:, l0 + s * P:l0 + (s + 1) * P],
                                            ident[:])
                    scrap = sb.tile([P, C], F32, tag="scrap")
                    nc.scalar.activation(scrap[:], xt_ps[:],
                                         mybir.ActivationFunctionType.Square,
                                         accum_out=ss[:, s:s + 1])
                    xt_sb = sb.tile([P, C], F32, tag="xts", bufs=6)
                    nc.vector.tensor_copy(xt_sb[:], xt_ps[:])
                    xts.append(xt_sb)

                # ---- inv norm [P, S] ----
                nrm = sb.tile([P, S], F32, tag="nrm")
                nc.scalar.sqrt(nrm[:], ss[:])
                nc.vector.tensor_scalar_max(nrm[:], nrm[:], EPS)
                inv = sb.tile([P, S], F32, tag="inv")
                nc.vector.reciprocal(inv[:], nrm[:])

                # ---- softmax over k (free dim) ----
                t = sb.tile([P, S * 64], F32, tag="t")
                tv = t[:].rearrange("p (s k) -> p s k", s=S)
                lgtv = lgt[:].rearrange("p (s k) -> p s k", s=S)
                nc.vector.tensor_mul(tv, lgtv, inv[:].broadcast_to([P, S, 64]))
                nc.vector.tensor_add(t[:], t[:], b_bc[0:P, 0:S * 64])
                negm = sb.tile([P, S], F32, tag="negm")
                nc.vector.tensor_reduce(negm[:], tv, axis=mybir.AxisListType.X,
                                        op=mybir.AluOpType.max, negate=True)
                e = sb.tile([P, S * 64], F32, tag="e")
                ev = e[:].rearrange("p (s k) -> p s k", s=S)
                nc.vector.tensor_add(ev, tv, negm[:].broadcast_to([P, S, 64]))
                a = sb.tile([P, S * 64], F32, tag="a")
                nc.scalar.activation(a[:], e[:], mybir.ActivationFunctionType.Exp)
                ssum = sb.tile([P, S], F32, tag="ssum")
                av = a[:].rearrange("p (s k) -> p s k", s=S)
                nc.vector.tensor_reduce(ssum[:], av, axis=mybir.AxisListType.X,
                                        op=mybir.AluOpType.add)
                rs = sb.tile([P, S], F32, tag="rs")
                nc.vector.reciprocal(rs[:], ssum[:])
                rsi = sb.tile([P, S], F32, tag="rsi")
                nc.vector.tensor_mul(rsi[:], rs[:], inv[:])
                ap_t = sb.tile([P, S * 64], F32, tag="ap")
                apv = ap_t[:].rearrange("p (s k) -> p s k", s=S)
                nc.vector.tensor_mul(apv, av, rsi[:].broadcast_to([P, S, 64]))

                # ---- vlad / asum accumulation (per-chunk groups) ----
                vlad_c = ps.tile([64, C], F32, tag="vlad", bufs=1,
                                 name=f"vlad{n}_{lc}")
                for s in range(S):
                    nc.tensor.matmul(vlad_c[:], ap_t[:, s * 64:(s + 1) * 64], xts[s][:],
                                     start=(s == 0), stop=(s == S - 1))
                asum_c = ps.tile([64, 1], F32, tag="asum", bufs=1,
                                 name=f"asum{n}_{lc}")
                for s in range(S):
                    nc.tensor.matmul(asum_c[:], a[:, s * 64:(s + 1) * 64], rs[:, s:s + 1],
                                     start=(s == 0), stop=(s == S - 1))
                if first:
                    nc.vector.tensor_copy(vacc[:], vlad_c[:])
                    nc.vector.tensor_copy(aacc[:], asum_c[:])
                else:
                    nc.vector.tensor_add(vacc[:], vacc[:], vlad_c[:])
                    nc.vector.tensor_add(aacc[:], aacc[:], asum_c[:])

            # ---- per-image epilogue ----
            cs = sb.tile([64, C], F32, tag="cs")
            nc.vector.tensor_scalar_mul(cs[:], cent[:], aacc[:])
            vl = sb.tile([64, C], F32, tag="vl")
            nc.vector.tensor_sub(vl[:], vacc[:], cs[:])
            scrap2 = sb.tile([64, C], F32, tag="scrap")
            ssq = sb.tile([64, 1], F32, tag="sm2")
            nc.scalar.activation(scrap2[:], vl[:],
                                 mybir.ActivationFunctionType.Square,
                                 accum_out=ssq[:])
            nrm2 = sb.tile([64, 1], F32, tag="sm3")
            nc.scalar.sqrt(nrm2[:], ssq[:])
            nc.vector.tensor_scalar_max(nrm2[:], nrm2[:], EPS)
            iv = sb.tile([64, 1], F32, tag="sm4")
            nc.vector.reciprocal(iv[:], nrm2[:])
            contrib = sb.tile([64, 1], F32, tag="sm5")
            nc.vector.tensor_mul(contrib[:], ssq[:], iv[:])
            nc.vector.tensor_mul(contrib[:], contrib[:], iv[:])
            tot = ps.tile([64, 1], F32, tag="asum", bufs=1, name=f"tot{n}")
            if not SKIP_TOT:
                nc.tensor.matmul(tot[:], ones64[:], contrib[:], start=True, stop=True)
            else:
                nc.vector.memset(tot[:], 1.0)
            gn = sb.tile([64, 1], F32, tag="sm6")
            nc.scalar.sqrt(gn[:], tot[:])
            nc.vector.tensor_scalar_max(gn[:], gn[:], EPS)
            gi = sb.tile([64, 1], F32, tag="sm7")
            nc.vector.reciprocal(gi[:], gn[:])
            sc = sb.tile([64, 1], F32, tag="sm8")
            nc.vector.tensor_mul(sc[:], iv[:], gi[:])
            outt = sb.tile([64, C], F32, tag="outt")
            nc.vector.tensor_scalar_mul(outt[:], vl[:], sc[:])
            yv = y_d[n:n + 1, :].rearrange("a (k c) -> (a k) c", c=C)
            nc.sync.dma_start(yv, outt[:])


def kernel(x, conv_w, conv_b, centroids, trace=False):
    if "nc" not in _CACHE:
        _CACHE["nc"] = _build()
    nc = _CACHE["nc"]
    x = np.ascontiguousarray(np.asarray(x, dtype=np.float32)).reshape(N, C, HW)
    in_maps = []
    for c in range(NCORES):
        in_maps.append({
            "x": x[c * NPC:(c + 1) * NPC],
            "conv_w": np.asarray(conv_w, dtype=np.float32),
            "conv_b": np.asarray(conv_b, dtype=np.float32).reshape(1, K),
            "centroids": np.asarray(centroids, dtype=np.float32),
        })
    res = bass_utils.run_bass_kernel_spmd(nc, in_maps, core_ids=list(range(NCORES)),
                                          trace=trace)
    out = np.concatenate([res.results[c]["y"] for c in range(NCORES)], axis=0)
    if trace:
        return out, res
    return out



# revision 19
# speedup vs baseline: 1.0007x; 1.0007x over previous
"""NetVLAD Trainium2 kernel: data-parallel over batch across 8 NeuronCores.

V3: bf16 matmul pipeline (1 cyc/row PE, fast DVE copies), logits computed
pre-transposed via x-stationary matmuls, whole-image softmax tiles,
vlad + asum accumulated fully in PSUM, PSUM-bank-collision-safe ordering,
software-pipelined PE stream (vlad/epilogue deferred by one image).
"""
import sys
sys.path.insert(0, "/opt/trn_rl_repo")
import os
import numpy as np
import concourse.bass as bass
import concourse.tile as tile
from concourse import bacc, mybir, masks
from concourse import bass_utils

F32 = mybir.dt.float32
BF16 = mybir.dt.bfloat16
N, C, HW, K = 32, 512, 1600, 64
NCORES = 8
NPC = N // NCORES          # images per core
EPS = 1e-12
CT = C // 128               # 4 c-tiles
NS = 13                     # l-segments: 12 x 128 + 1 x 64
_CACHE = {}


def _build():
    nc = bacc.Bacc("TRN2", target_bir_lowering=False, debug=False,
                   num_devices=NCORES)
    x_d = nc.dram_tensor("x", [NPC, C, HW], F32, kind="ExternalInput")
    w_d = nc.dram_tensor("conv_w", [K, C], F32, kind="ExternalInput")
    b_d = nc.dram_tensor("conv_b", [1, K], F32, kind="ExternalInput")
    c_d = nc.dram_tensor("centroids", [K, C], F32, kind="ExternalInput")
    y_d = nc.dram_tensor("y", [NPC, K * C], F32, kind="ExternalOutput")

    with tile.TileContext(nc) as tc:
        _emit(nc, tc, x_d, w_d, b_d, c_d, y_d)
    nc.finalize()
    return nc


def _emit(nc, tc, x_d, w_d, b_d, c_d, y_d):
    pool = nc.vector if os.environ.get("NOPOOL") else nc.gpsimd
    import contextlib
    ctx = contextlib.ExitStack()
    AF = mybir.ActivationFunctionType
    AX = mybir.AxisListType
    OP = mybir.AluOpType
    ctxm = {}

    with ctx:
        const = ctx.enter_context(tc.tile_pool(name="const", bufs=1))
        xp = ctx.enter_context(tc.tile_pool(name="xp", bufs=2))
        sb = ctx.enter_context(tc.tile_pool(name="sb", bufs=2))
        ps = ctx.enter_context(tc.tile_pool(name="ps", bufs=2, space="PSUM"))

        ident = const.tile([128, 128], F32)
        masks.make_identity(nc, ident[:])
        ident_b = const.tile([128, 128], BF16)
        pool.tensor_copy(ident_b[:], ident[:])
        ones64 = const.tile([64, 64], BF16)
        pool.memset(ones64[:], 1.0)
        cent = const.tile([64, C], F32)
        nc.sync.dma_start(cent[:], c_d[:, :])
        b_bc = const.tile([128, NS * 64], F32)
        for s in range(NS):
            nc.sync.dma_start(b_bc[:, s * 64:(s + 1) * 64],
                              b_d[0:1, :].broadcast_to([128, K]))
        # conv_w^T in bf16: [128 (c within tile), 4ct * 64k]
        w_sb = const.tile([64, C], F32)
        nc.sync.dma_start(w_sb[:], w_d[:, :])
        wT = const.tile([128, CT * 64], BF16)
        wt_ps = ps.tile([128, 832], F32, tag="lgt", bufs=1, name="wt_ps")
        for ct in range(CT):
            nc.tensor.transpose(wt_ps[:, ct * 64:(ct + 1) * 64],
                                w_sb[:, ct * 128:(ct + 1) * 128],
                                ident[0:64, 0:64])
        nc.vector.tensor_copy(wT[:], wt_ps[:, 0:CT * 64])
        scrap_b = const.tile([128, 512], BF16, name="scrapb")
        scrap_d = const.tile([128, 512], BF16, name="scrapd")

        def front(n):
            m = {}
            # ---- load x f32 [128, 4ct*1600], convert to bf16 ----
            xs = xp.tile([128, CT * HW], F32, tag="x", name=f"x_{n}")
            for ct in range(CT):
                nc.sync.dma_start(xs[:, ct * HW:(ct + 1) * HW],
                                  x_d[n, ct * 128:(ct + 1) * 128, :])
            xb = xp.tile([128, CT * HW + 64], BF16, tag="xb", name=f"xb_{n}")
            pool.memset(xb[:, CT * HW:CT * HW + 64], 0.0)
            xs4 = xs[:].rearrange("p (c l) -> p c l", c=CT)
            xb4 = xb[:, 0:CT * HW].rearrange("p (c l) -> p c l", c=CT)
            for q in range(4):
                pool.tensor_copy(xb4[:, :, q * 400:(q + 1) * 400],
                                      xs4[:, :, q * 400:(q + 1) * 400])

            # ---- logits^T [128l, 64k] per seg (x-stationary) interleaved
            #      with x^T transpose groups ----
            lgT = ps.tile([128, 832], F32, tag="lgt", bufs=1, name=f"lgt{n}")
            xt_sb = sb.tile([128, NS * 512], BF16, tag="xts", name=f"xts{n}")
            ss = sb.tile([128, NS], F32, tag="ss", name=f"ss{n}")
            pool.memset(ss[64:128, 12:13], 1.0)
            pool.memset(xt_sb[64:128, 12 * 512:13 * 512], 0.0)

            def lg_segs(s0, s1):
                # tail seg reads 64 cols past the image into pad/next-ct
                # (garbage rows 64-127 are neutralized after softmax)
                for s in range(s0, s1):
                    for ct in range(CT):
                        nc.tensor.matmul(
                            lgT[:, s * 64:(s + 1) * 64],
                            xb[:, ct * HW + s * 128:ct * HW + (s + 1) * 128],
                            wT[:, ct * 64:(ct + 1) * 64],
                            start=(ct == 0), stop=(ct == CT - 1),
                            skip_group_check=True)

            def xt_group(g):
                xtg = ps.tile([128, 2048], BF16, tag="xt", bufs=1,
                              name=f"xt{n}_{g}")
                for i in range(4):
                    s = g * 4 + i
                    for ct in range(CT):
                        nc.tensor.transpose(
                            xtg[:, i * 512 + ct * 128:i * 512 + (ct + 1) * 128],
                            xb[:, ct * HW + s * 128:ct * HW + (s + 1) * 128],
                            ident_b[:])
                eng = nc.vector if g % 2 == 0 else nc.scalar
                if g % 2 == 0:
                    nc.vector.tensor_copy(xt_sb[:, g * 2048:(g + 1) * 2048],
                                          xtg[:])
                else:
                    nc.scalar.copy(xt_sb[:, g * 2048:(g + 1) * 2048], xtg[:])

            lg_segs(0, 4)
            xt_group(0)
            lg_segs(4, 8)
            xt_group(1)
            lg_segs(8, 13)
            xt_group(2)
            xtl = ps.tile([64, 512], BF16, tag="xtl", bufs=1, name=f"xtl{n}")
            for ct in range(CT):
                nc.tensor.transpose(
                    xtl[:, ct * 128:(ct + 1) * 128],
                    xb[:, ct * HW + 12 * 128:ct * HW + 12 * 128 + 64],
                    ident_b[:])
            nc.vector.tensor_copy(xt_sb[0:64, 12 * 512:13 * 512], xtl[:])

            # ---- sumsq from SBUF copies (collision-safe; ttr-bf16 crashes HW
            #      so all segments go through Act Square) ----
            for s in range(NS):
                P = 128 if s < 12 else 64
                nc.scalar.activation(scrap_b[0:P, :],
                                     xt_sb[0:P, s * 512:(s + 1) * 512],
                                     AF.Square, accum_out=ss[0:P, s:s + 1])

            # ---- norms ----
            nrm = sb.tile([128, NS], F32, tag="nrm", name=f"nrm{n}")
            nc.scalar.sqrt(nrm[:], ss[:])
            pool.tensor_scalar_max(nrm[:], nrm[:], EPS)
            inv = sb.tile([128, NS], F32, tag="inv", name=f"inv{n}")
            nc.vector.reciprocal(inv[:], nrm[:])
            # norms at even cols (4B-aligned slices for the PE), zeros between
            nrm_b = sb.tile([128, 2 * NS], BF16, tag="nrmb", name=f"nrmb{n}")
            pool.memset(nrm_b[:], 0.0)
            nrm_b2 = nrm_b[:].rearrange("p (s two) -> p s two", two=2)
            pool.tensor_copy(nrm_b2[:, :, 0:1], nrm[:].unsqueeze(-1))

            # ---- softmax over k for the whole image [128, 13, 64] ----
            t = sb.tile([128, NS * 64], F32, tag="t", name=f"t{n}")
            t3 = t[:].rearrange("p (s k) -> p s k", s=NS)
            lgt3 = lgT[:, 0:NS * 64].rearrange("p (s k) -> p s k", s=NS)
            nc.vector.tensor_mul(t3, lgt3, inv[:].broadcast_to([128, NS, 64]))
            pool.tensor_add(t[:], t[:], b_bc[:])
            negm = sb.tile([128, NS], F32, tag="negm", name=f"negm{n}")
            nc.vector.tensor_reduce(negm[:], t3, axis=AX.X, op=OP.max,
                                    negate=True)
            pool.tensor_add(t3, t3, negm[:].broadcast_to([128, NS, 64]))
            a = sb.tile([128, NS * 64], BF16, tag="a", name=f"a{n}")
            nc.scalar.activation(a[:], t[:], AF.Exp)
            a3 = a[:].rearrange("p (s k) -> p s k", s=NS)
            ssum = sb.tile([128, NS], F32, tag="ssum", name=f"ssum{n}")
            nc.vector.tensor_reduce(ssum[:], a3, axis=AX.X, op=OP.add)
            rsi = sb.tile([128, NS], F32, tag="rsi", name=f"rsi{n}")
            nc.vector.reciprocal(rsi[:], ssum[:])
            nc.vector.tensor_mul(rsi[:], rsi[:], inv[:])
            rsi_b = sb.tile([128, NS], BF16, tag="rsib", name=f"rsib{n}")
            pool.tensor_copy(rsi_b[:], rsi[:])
            ap_t = sb.tile([128, NS * 64], BF16, tag="ap", name=f"ap{n}")
            ap3 = ap_t[:].rearrange("p (s k) -> p s k", s=NS)
            pool.tensor_mul(ap3, a3, rsi_b[:].broadcast_to([128, NS, 64]))
            pool.memset(ap_t[64:128, 12 * 64:13 * 64], 0.0)
            m["xt_sb"], m["nrm_b"], m["ap_t"] = xt_sb, nrm_b, ap_t
            return m

        def back(n, m):
            xt_sb, nrm_b, ap_t = m["xt_sb"], m["nrm_b"], m["ap_t"]
            # ---- vlad [64, 512] + asum [64, 1] accumulated in PSUM ----
            vlad = ps.tile([64, 512], F32, tag="vlad", bufs=1, name=f"vlad{n}")
            asum = ps.tile([64, 2], F32, tag="as", name=f"as{n}")
            for s in range(NS):
                nc.tensor.matmul(vlad[:], ap_t[:, s * 64:(s + 1) * 64],
                                 xt_sb[:, s * 512:(s + 1) * 512],
                                 start=(s == 0), stop=(s == NS - 1))
                # ap . ||x|| = softmax/||x|| * ||x|| = plain softmax sum
                # (col 1 of the rhs pair is zero padding -> asum col 1 is 0)
                nc.tensor.matmul(asum[:], ap_t[:, s * 64:(s + 1) * 64],
                                 nrm_b[:, 2 * s:2 * s + 2],
                                 start=(s == 0), stop=(s == NS - 1))

            # ---- per-image epilogue ----
            aacc = sb.tile([64, 1], F32, tag="aacc", name=f"aacc{n}")
            nc.scalar.copy(aacc[:], asum[:, 0:1])
            cs = sb.tile([64, C], F32, tag="cs", name=f"cs{n}")
            nc.scalar.mul(cs[:], cent[:], aacc[:])
            vl = sb.tile([64, C], F32, tag="vl", name=f"vl{n}")
            nc.vector.tensor_sub(vl[:], vlad[:], cs[:])
            scrap2 = sb.tile([64, C], F32, tag="scrap2", name=f"scr2{n}")
            ssq = sb.tile([64, 8], F32, tag="sm", name=f"sm{n}")
            nc.scalar.activation(scrap2[:], vl[:], AF.Square,
                                 accum_out=ssq[:, 0:1])
            nc.scalar.sqrt(ssq[:, 1:2], ssq[:, 0:1])
            pool.tensor_scalar_max(ssq[:, 1:2], ssq[:, 1:2], EPS)
            nc.vector.reciprocal(ssq[:, 2:3], ssq[:, 1:2])  # iv
            pool.tensor_mul(ssq[:, 3:4], ssq[:, 0:1], ssq[:, 2:3])
            pool.tensor_mul(ssq[:, 3:4], ssq[:, 3:4], ssq[:, 2:3])
            contrib_b = sb.tile([64, 2], BF16, tag="ctb", name=f"ctb{n}")
            pool.memset(contrib_b[:, 1:2], 0.0)
            pool.tensor_copy(contrib_b[:, 0:1], ssq[:, 3:4])
            # tot reuses the asum slot (WAR on aacc read serializes it)
            nc.tensor.matmul(asum[:], ones64[:], contrib_b[:],
                             start=True, stop=True, skip_group_check=True)
            nc.scalar.sqrt(ssq[:, 4:5], asum[:, 0:1])
            pool.tensor_scalar_max(ssq[:, 4:5], ssq[:, 4:5], EPS)
            nc.vector.reciprocal(ssq[:, 5:6], ssq[:, 4:5])  # gi
            pool.tensor_mul(ssq[:, 6:7], ssq[:, 2:3], ssq[:, 5:6])
            outt = sb.tile([64, C], F32, tag="outt", name=f"outt{n}")
            pool.tensor_scalar_mul(outt[:], vl[:], ssq[:, 6:7])
            yv = y_d[n:n + 1, :].rearrange("a (k c) -> (a k) c", c=C)
            nc.sync.dma_start(yv, outt[:])

        for n in range(NPC + 1):
            if n < NPC:
                ctxm[n] = front(n)
            if n >= 1:
                back(n - 1, ctxm.pop(n - 1))


def kernel(x, conv_w, conv_b, centroids, trace=False):
    if "nc" not in _CACHE:
        _CACHE["nc"] = _build()
    nc = _CACHE["nc"]
    x = np.ascontiguousarray(np.asarray(x, dtype=np.float32)).reshape(N, C, HW)
    in_maps = []
    for c in range(NCORES):
        in_maps.append({
            "x": x[c * NPC:(c + 1) * NPC],
            "conv_w": np.asarray(conv_w, dtype=np.float32),
            "conv_b": np.asarray(conv_b, dtype=np.float32).reshape(1, K),
            "centroids": np.asarray(centroids, dtype=np.float32),
        })
    res = bass_utils.run_bass_kernel_spmd(nc, in_maps, core_ids=list(range(NCORES)),
                                          trace=trace)
    out = np.concatenate([res.results[c]["y"] for c in range(NCORES)], axis=0)
    if trace:
        return out, res
    return out


# revision 20
# speedup vs baseline: 1.1046x; 1.1038x over previous
"""NetVLAD Trainium2 kernel: data-parallel over batch across 8 NeuronCores.

V4: bf16 matmul pipeline; two-term (hi+lo) conv_w split for precision;
logits computed pre-transposed via x-stationary matmuls; whole-image softmax;
vlad + asum accumulated fully in PSUM; engine assignment tuned to measured
rates (Pool only for rare big ops - it has ~1.3us per-instruction overhead).
"""
import sys
sys.path.insert(0, "/opt/trn_rl_repo")
import os
import numpy as np
import concourse.bass as bass
import concourse.tile as tile
from concourse import bacc, mybir, masks
from concourse import bass_utils

F32 = mybir.dt.float32
BF16 = mybir.dt.bfloat16
N, C, HW, K = 32, 512, 1600, 64
NCORES = 8
NPC = N // NCORES          # images per core
EPS = 1e-12
CT = C // 128               # 4 c-tiles
NS = 13                     # l-segments: 12 x 128 + 1 x 64
ACT_SS = int(os.environ.get("ACT_SS", "9"))   # segs of sumsq on Act engine
WLO = os.environ.get("WLO", "1") == "1"       # two-term w split
_CACHE = {}


def _build():
    nc = bacc.Bacc("TRN2", target_bir_lowering=False, debug=False,
                   num_devices=NCORES)
    x_d = nc.dram_tensor("x", [NPC, C, HW], F32, kind="ExternalInput")
    w_d = nc.dram_tensor("conv_w", [K, C], F32, kind="ExternalInput")
    b_d = nc.dram_tensor("conv_b", [1, K], F32, kind="ExternalInput")
    c_d = nc.dram_tensor("centroids", [K, C], F32, kind="ExternalInput")
    y_d = nc.dram_tensor("y", [NPC, K * C], F32, kind="ExternalOutput")

    with tile.TileContext(nc) as tc:
        _emit(nc, tc, x_d, w_d, b_d, c_d, y_d)
    nc.finalize()
    return nc


def _emit(nc, tc, x_d, w_d, b_d, c_d, y_d):
    import contextlib
    ctx = contextlib.ExitStack()
    AF = mybir.ActivationFunctionType
    AX = mybir.AxisListType
    OP = mybir.AluOpType
    ctxm = {}

    with ctx:
        const = ctx.enter_context(tc.tile_pool(name="const", bufs=1))
        xp = ctx.enter_context(tc.tile_pool(name="xp", bufs=2))
        sb = ctx.enter_context(tc.tile_pool(name="sb", bufs=2))
        ps = ctx.enter_context(tc.tile_pool(name="ps", bufs=2, space="PSUM"))

        ident = const.tile([128, 128], F32)
        masks.make_identity(nc, ident[:])
        ident_b = const.tile([128, 128], BF16)
        nc.vector.tensor_copy(ident_b[:], ident[:])
        ones64 = const.tile([64, 64], BF16)
        nc.vector.memset(ones64[:], 1.0)
        cent = const.tile([64, C], F32)
        nc.sync.dma_start(cent[:], c_d[:, :])
        b_bc = const.tile([128, NS * 64], F32)
        for s in range(NS):
            nc.sync.dma_start(b_bc[:, s * 64:(s + 1) * 64],
                              b_d[0:1, :].broadcast_to([128, K]))
        # conv_w^T split into bf16 hi + lo parts: [128 (c in tile), 4ct*64k]
        w_sb = const.tile([64, C], F32)
        nc.sync.dma_start(w_sb[:], w_d[:, :])
        wT = const.tile([128, CT * 64], BF16)
        wT_lo = const.tile([128, CT * 64], BF16)
        wt_ps = ps.tile([128, 832], F32, tag="lgt", bufs=1, name="wt_ps")
        for ct in range(CT):
            nc.tensor.transpose(wt_ps[:, ct * 64:(ct + 1) * 64],
                                w_sb[:, ct * 128:(ct + 1) * 128],
                                ident[0:64, 0:64])
        nc.vector.tensor_copy(wT[:], wt_ps[:, 0:CT * 64])
        wres = const.tile([128, CT * 64], F32)
        nc.vector.tensor_sub(wres[:], wt_ps[:, 0:CT * 64], wT[:])
        nc.vector.tensor_copy(wT_lo[:], wres[:])
        scrap_b = const.tile([128, 512], BF16, name="scrapb")
        sq_b = const.tile([128, (NS - ACT_SS) * 512], BF16, name="sqb")

        def front(n):
            m = {}
            # ---- load x f32, convert to bf16 (one Pool instruction) ----
            xs = xp.tile([128, CT * HW], F32, tag="x", name=f"x_{n}")
            for ct in range(CT):
                nc.sync.dma_start(xs[:, ct * HW:(ct + 1) * HW],
                                  x_d[n, ct * 128:(ct + 1) * 128, :])
            xb = xp.tile([128, CT * HW + 64], BF16, tag="xb", name=f"xb_{n}")
            nc.vector.memset(xb[:, CT * HW:CT * HW + 64], 0.0)
            nc.gpsimd.tensor_copy(xb[:, 0:CT * HW], xs[:])

            lgT = ps.tile([128, 832], F32, tag="lgt", bufs=1, name=f"lgt{n}")
            xt_sb = sb.tile([128, NS * 512], BF16, tag="xts", name=f"xts{n}")
            ss = sb.tile([128, NS], F32, tag="ss", name=f"ss{n}")
            nc.vector.memset(xt_sb[64:128, 12 * 512:13 * 512], 0.0)

            def lg_segs(s0, s1):
                # tail seg reads 64 cols past the image into pad/next-ct
                # (garbage rows 64-127 are neutralized after softmax)
                for s in range(s0, s1):
                    for ct in range(CT):
                        nc.tensor.matmul(
                            lgT[:, s * 64:(s + 1) * 64],
                            xb[:, ct * HW + s * 128:ct * HW + (s + 1) * 128],
                            wT[:, ct * 64:(ct + 1) * 64],
                            start=(ct == 0), stop=(not WLO and ct == CT - 1),
                            skip_group_check=True)
                    if WLO:
                        for ct in range(CT):
                            nc.tensor.matmul(
                                lgT[:, s * 64:(s + 1) * 64],
                                xb[:, ct * HW + s * 128:ct * HW + (s + 1) * 128],
                                wT_lo[:, ct * 64:(ct + 1) * 64],
                                start=False, stop=(ct == CT - 1),
                                skip_group_check=True)

            def xt_group(g):
                xtg = ps.tile([128, 2048], BF16, tag="xt", bufs=1,
                              name=f"xt{n}_{g}")
                for i in range(4):
                    s = g * 4 + i
                    for ct in range(CT):
                        nc.tensor.transpose(
                            xtg[:, i * 512 + ct * 128:i * 512 + (ct + 1) * 128],
                            xb[:, ct * HW + s * 128:ct * HW + (s + 1) * 128],
                            ident_b[:])
                nc.vector.tensor_copy(xt_sb[:, g * 2048:(g + 1) * 2048],
                                      xtg[:])

            lg_segs(0, 4)
            xt_group(0)
            lg_segs(4, 8)
            xt_group(1)
            lg_segs(8, 13)
            xt_group(2)
            xtl = ps.tile([64, 512], BF16, tag="xtl", bufs=1, name=f"xtl{n}")
            for ct in range(CT):
                nc.tensor.transpose(
                    xtl[:, ct * 128:(ct + 1) * 128],
                    xb[:, ct * HW + 12 * 128:ct * HW + 12 * 128 + 64],
                    ident_b[:])
            nc.vector.tensor_copy(xt_sb[0:64, 12 * 512:13 * 512], xtl[:])

            # ---- sumsq: first ACT_SS segs on Act, rest on DVE (TT+reduce) --
            for s in range(ACT_SS):
                P = 128 if s < 12 else 64
                nc.scalar.activation(scrap_b[0:P, :],
                                     xt_sb[0:P, s * 512:(s + 1) * 512],
                                     AF.Square, accum_out=ss[0:P, s:s + 1])
            nd = NS - ACT_SS
            nc.vector.tensor_mul(sq_b[:], xt_sb[:, ACT_SS * 512:NS * 512],
                                 xt_sb[:, ACT_SS * 512:NS * 512])
            sq3 = sq_b[:].rearrange("p (s c) -> p s c", s=nd)
            nc.vector.tensor_reduce(ss[:, ACT_SS:NS], sq3, axis=AX.X,
                                    op=OP.add)

            # ---- norms ----
            nrm = sb.tile([128, NS], F32, tag="nrm", name=f"nrm{n}")
            nc.scalar.sqrt(nrm[:], ss[:])
            nc.vector.tensor_scalar_max(nrm[:], nrm[:], EPS)
            inv = sb.tile([128, NS], F32, tag="inv", name=f"inv{n}")
            nc.vector.reciprocal(inv[:], nrm[:])
            # norms at even cols (4B-aligned slices for the PE), zeros between
            nrm_b = sb.tile([128, 2 * NS], BF16, tag="nrmb", name=f"nrmb{n}")
            nc.vector.memset(nrm_b[:], 0.0)
            nrm_b2 = nrm_b[:].rearrange("p (s two) -> p s two", two=2)
            nc.vector.tensor_copy(nrm_b2[:, :, 0:1], nrm[:].unsqueeze(-1))

            # ---- softmax over k for the whole image [128, 13, 64] ----
            t = sb.tile([128, NS * 64], F32, tag="t", name=f"t{n}")
            t3 = t[:].rearrange("p (s k) -> p s k", s=NS)
            lgt3 = lgT[:, 0:NS * 64].rearrange("p (s k) -> p s k", s=NS)
            nc.vector.tensor_mul(t3, lgt3, inv[:].broadcast_to([128, NS, 64]))
            nc.vector.tensor_add(t[:], t[:], b_bc[:])
            negm = sb.tile([128, NS], F32, tag="negm", name=f"negm{n}")
            nc.vector.tensor_reduce(negm[:], t3, axis=AX.X, op=OP.max,
                                    negate=True)
            nc.vector.tensor_add(t3, t3, negm[:].broadcast_to([128, NS, 64]))
            a = sb.tile([128, NS * 64], BF16, tag="a", name=f"a{n}")
            nc.scalar.activation(a[:], t[:], AF.Exp)
            a3 = a[:].rearrange("p (s k) -> p s k", s=NS)
            ssum = sb.tile([128, NS], F32, tag="ssum", name=f"ssum{n}")
            nc.vector.tensor_reduce(ssum[:], a3, axis=AX.X, op=OP.add)
            rsi = sb.tile([128, NS], F32, tag="rsi", name=f"rsi{n}")
            nc.vector.reciprocal(rsi[:], ssum[:])
            nc.vector.tensor_mul(rsi[:], rsi[:], inv[:])
            rsi_b = sb.tile([128, NS], BF16, tag="rsib", name=f"rsib{n}")
            nc.vector.tensor_copy(rsi_b[:], rsi[:])
            ap_t = sb.tile([128, NS * 64], BF16, tag="ap", name=f"ap{n}")
            ap3 = ap_t[:].rearrange("p (s k) -> p s k", s=NS)
            nc.gpsimd.tensor_mul(ap3, a3, rsi_b[:].broadcast_to([128, NS, 64]))
            # tail segment: rows 64-127 are garbage -> zero them
            nc.vector.memset(ap_t[64:128, 12 * 64:13 * 64], 0.0)
            m["xt_sb"], m["nrm_b"], m["ap_t"] = xt_sb, nrm_b, ap_t
            return m

        def back(n, m):
            xt_sb, nrm_b, ap_t = m["xt_sb"], m["nrm_b"], m["ap_t"]
            # ---- vlad [64, 512] + asum [64, 1] accumulated in PSUM ----
            vlad = ps.tile([64, 512], F32, tag="vlad", bufs=1, name=f"vlad{n}")
            asum = ps.tile([64, 2], F32, tag="as", bufs=1, name=f"as{n}")
            for s in range(NS):
                nc.tensor.matmul(vlad[:], ap_t[:, s * 64:(s + 1) * 64],
                                 xt_sb[:, s * 512:(s + 1) * 512],
                                 start=(s == 0), stop=(s == NS - 1))
                # ap . ||x|| = softmax/||x|| * ||x|| = plain softmax sum
                nc.tensor.matmul(asum[:], ap_t[:, s * 64:(s + 1) * 64],
                                 nrm_b[:, 2 * s:2 * s + 2],
                                 start=(s == 0), stop=(s == NS - 1))

            # ---- per-image epilogue ----
            aacc = sb.tile([64, 1], F32, tag="aacc", name=f"aacc{n}")
            nc.scalar.copy(aacc[:], asum[:, 0:1])
            cs = sb.tile([64, C], F32, tag="cs", name=f"cs{n}")
            nc.scalar.mul(cs[:], cent[:], aacc[:])
            vl = sb.tile([64, C], F32, tag="vl", name=f"vl{n}")
            nc.vector.tensor_sub(vl[:], vlad[:], cs[:])
            scrap2 = sb.tile([64, C], F32, tag="scrap2", name=f"scr2{n}")
            ssq = sb.tile([64, 8], F32, tag="sm", name=f"sm{n}")
            nc.scalar.activation(scrap2[:], vl[:], AF.Square,
                                 accum_out=ssq[:, 0:1])
            nc.scalar.sqrt(ssq[:, 1:2], ssq[:, 0:1])
            nc.vector.tensor_scalar_max(ssq[:, 1:2], ssq[:, 1:2], EPS)
            nc.vector.reciprocal(ssq[:, 2:3], ssq[:, 1:2])  # iv
            nc.vector.tensor_mul(ssq[:, 3:4], ssq[:, 0:1], ssq[:, 2:3])
            nc.vector.tensor_mul(ssq[:, 3:4], ssq[:, 3:4], ssq[:, 2:3])
            contrib_b = sb.tile([64, 2], BF16, tag="ctb", name=f"ctb{n}")
            nc.vector.memset(contrib_b[:, 1:2], 0.0)
            nc.vector.tensor_copy(contrib_b[:, 0:1], ssq[:, 3:4])
            # tot reuses the asum slot (WAR on aacc read serializes it)
            nc.tensor.matmul(asum[:], ones64[:], contrib_b[:],
                             start=True, stop=True, skip_group_check=True)
            nc.scalar.sqrt(ssq[:, 4:5], asum[:, 0:1])
            nc.vector.tensor_scalar_max(ssq[:, 4:5], ssq[:, 4:5], EPS)
            nc.vector.reciprocal(ssq[:, 5:6], ssq[:, 4:5])  # gi
            nc.vector.tensor_mul(ssq[:, 6:7], ssq[:, 2:3], ssq[:, 5:6])
            outt = sb.tile([64, C], F32, tag="outt", name=f"outt{n}")
            nc.scalar.mul(outt[:], vl[:], ssq[:, 6:7])
            yv = y_d[n:n + 1, :].rearrange("a (k c) -> (a k) c", c=C)
            nc.sync.dma_start(yv, outt[:])

        for n in range(NPC + 1):
            if n < NPC:
                ctxm[n] = front(n)
            if n >= 1:
                back(n - 1, ctxm.pop(n - 1))


def kernel(x, conv_w, conv_b, centroids, trace=False):
    if "nc" not in _CACHE:
        _CACHE["nc"] = _build()
    nc = _CACHE["nc"]
    x = np.ascontiguousarray(np.asarray(x, dtype=np.float32)).reshape(N, C, HW)
    in_maps = []
    for c in range(NCORES):
        in_maps.append({
            "x": x[c * NPC:(c + 1) * NPC],
            "conv_w": np.asarray(conv_w, dtype=np.float32),
            "conv_b": np.asarray(conv_b, dtype=np.float32).reshape(1, K),
            "centroids": np.asarray(centroids, dtype=np.float32),
        })
    res = bass_utils.run_bass_kernel_spmd(nc, in_maps, core_ids=list(range(NCORES)),
                                          trace=trace)
    out = np.concatenate([res.results[c]["y"] for c in range(NCORES)], axis=0)
    if trace:
        return out, res
    return out
